# revision 12
# baseline (speedup 1.0000x reference)
"""Trainium2 Bass kernel for nn_DBLoss_11605001634022.

DBLoss = Ls + Lb + 10*Lt over four (16,640,640) f32 maps, where Ls/Lb are
"balanced" BCE-with-logits losses with hard-negative mining (keep the top
n_negative = min(n_neg_avail, 3*n_pos) negative losses) and
Lt = mean|thresh - target_thresh|.

For these inputs the targets are ~uniform, so n_neg_avail <= 3*n_pos by a
huge margin and the top-k keeps ALL negatives; each balanced BCE collapses
to a plain mean of the elementwise BCE losses. With
bce(x, t) = softplus(x) - x*t, the whole loss is one streaming reduction:

  loss = [ S(sp(p)) - S(p*tp) + S(sp(50*a)) - 2500*S(a*b) + 10*S(|c|) ] / N
  a = p - t,  b = tp - tt,  c = t - tt,   S = sum over all elements

The kernel verifies the collapse condition on the host (cheap) and falls
back to an exact numpy implementation if it ever fails.

The HW has no softplus ACT table, so softplus uses the relu identity
  S(sp(x)) = (S(x) + S(|x|))/2 + S(ln(1 + exp(-|x|)))
with exp/ln in the single `natural_log_exp_and_others` ACT table set (one
table load, no switches). Likewise S(|c|) = 2 S(relu(c)) - S(t) + S(tt).

Sharded batch-parallel: 2 images/core across 8 cores; each core streams
its 13.1 MB once, in 4 double-buffered [128,1600] chunks, every engine
loaded at/below the ~7.7 us/chunk DMA cadence (raw Bass + manual
semaphores; the Tile layer's multi-wait sync is rejected by this walrus):
  POOL (3 subs): a=p-t, b=tp-tt, c=t-tt.
  DVE  (5 ops): nap=(p*-1) min p = -|p| (+free row-sum); fused
        multiply+row-sum (p*-1)*tp and (a*-2500)*b; row-sums of p and
        relu(c) via tensor_scalar cache-reduce.
  ACT  (4 passes): |a| (+row-sum); exp(-50|a|) and exp(-|p|) into one
        [128,3200] buffer; single merged ln(1+u) pass (+row-sum).
  PE   (8 fp32 ones-matmuls): column sums of t and tt accumulated in
        PSUM across chunks (exact, PE idle otherwise).
Row-sums land in per-engine stats tiles (no cross-engine SBUF write
granule sharing); PSUM column sums staged to SBUF at the end. Host applies
coefficients and the final division in float64.
"""

import numpy as np

N_CORES = 8
SHAPE = (16, 640, 640)
NTOT = SHAPE[0] * SHAPE[1] * SHAPE[2]
PER_CORE = NTOT // N_CORES  # 819200
P = 128
FDIM = PER_CORE // P  # 6400
NCHUNK = 4
F = FDIM // NCHUNK  # 1600
R = 50.0
ALPHA = 1.0
BETA = 10.0
K = 3

_CACHE = {}


def _get_concourse():
    try:
        import concourse.bass  # noqa: F401
    except ImportError:
        import sys

        sys.path.insert(0, "/opt/trn_rl_repo")
    import concourse.bass as bass
    import concourse.mybir as mybir
    from concourse import bass_utils

    return bass, mybir, bass_utils


def _build(nloop=1):
    """Build the bass program. nloop > 1 repeats the whole pipeline nloop
    times inside one NEFF (same result; used for dispatch-free timing)."""
    if nloop in _CACHE:
        return _CACHE[nloop]
    import contextlib

    bass, mybir, bass_utils = _get_concourse()
    f32 = mybir.dt.float32
    Alu = mybir.AluOpType
    Act = mybir.ActivationFunctionType

    nc = bass.Bass()
    dp = nc.dram_tensor("p", [P, FDIM], f32, kind="ExternalInput")
    dt_ = nc.dram_tensor("t", [P, FDIM], f32, kind="ExternalInput")
    dtp = nc.dram_tensor("tp", [P, FDIM], f32, kind="ExternalInput")
    dtt = nc.dram_tensor("tt", [P, FDIM], f32, kind="ExternalInput")
    # acc_out: [0:20] DVE chunk-major (stt1, stt2, nap, sump, reluc),
    # [20:28] ACT chunk-major (absA, lnC)
    dout = nc.dram_tensor("acc_out", [P, 7 * NCHUNK], f32, kind="ExternalOutput")
    dout2 = nc.dram_tensor("colsum_out", [1, 1024], f32, kind="ExternalOutput")

    NB = 2
    KSL = [(0, 512), (512, 1024), (1024, 1536), (1536, 1600)]
    T = nloop * NCHUNK

    ctx = contextlib.ExitStack()
    with ctx:
        sb = lambda name, shape: ctx.enter_context(
            nc.sbuf_tensor(name, shape, f32)
        )
        tP = [sb(f"tP{i}", [P, F]) for i in range(NB)]
        tT = [sb(f"tT{i}", [P, F]) for i in range(NB)]
        tTP = [sb(f"tTP{i}", [P, F]) for i in range(NB)]
        tTT = [sb(f"tTT{i}", [P, F]) for i in range(NB)]
        tA = [sb(f"tA{i}", [P, F]) for i in range(NB)]
        tB = [sb(f"tB{i}", [P, F]) for i in range(NB)]
        tC = [sb(f"tC{i}", [P, F]) for i in range(NB)]
        tNP = [sb(f"tNP{i}", [P, F]) for i in range(NB)]
        tAA = sb("tAA", [P, F])
        tE = sb("tE", [P, 2 * F])  # exp outputs (p-half | a-half)
        tF = sb("tF", [P, 2 * F])  # ln dump
        trash = sb("trash", [P, F])
        acc_d = sb("acc_d", [P, 5 * NCHUNK])  # stt1, stt2, nap, sump, reluc
        acc_a = sb("acc_a", [P, 2 * NCHUNK])  # absA, lnC per chunk
        csum = sb("csum", [1, 1024])
        psum_t = ctx.enter_context(nc.psum_tensor("psum_t", [1, 512], f32))
        psum_tt = ctx.enter_context(nc.psum_tensor("psum_tt", [1, 512], f32))

        ones = nc.const_aps.tensor(1.0, (P, 1), f32)

        dma_sem = ctx.enter_context(nc.semaphore())
        dve_sem = ctx.enter_context(nc.semaphore())
        act_sem = ctx.enter_context(nc.semaphore())
        pool_sem = ctx.enter_context(nc.semaphore())
        pe_sem = ctx.enter_context(nc.semaphore())
        block = ctx.enter_context(nc.Block())

        def dcol(j, k):
            return acc_d[:, 5 * j + k : 5 * j + k + 1]

        def acol(j, k):
            return acc_a[:, 2 * j + k : 2 * j + k + 1]

        @block.sync
        def _(sync):
            for jj in range(T):
                j = jj % NCHUNK
                bi = jj % NB
                sl = slice(j * F, (j + 1) * F)
                if jj >= NB:
                    # input buffers of chunk jj-2 must be fully consumed
                    sync.wait_ge(dve_sem, 5 * (jj - 1))
                    sync.wait_ge(pool_sem, 3 * (jj - 1))
                    sync.wait_ge(pe_sem, 8 * (jj - 1))
                sync.dma_start(out=tP[bi][:], in_=dp[:, sl]).then_inc(dma_sem, 16)
                sync.dma_start(out=tT[bi][:], in_=dt_[:, sl]).then_inc(dma_sem, 16)
                sync.dma_start(out=tTP[bi][:], in_=dtp[:, sl]).then_inc(dma_sem, 16)
                sync.dma_start(out=tTT[bi][:], in_=dtt[:, sl]).then_inc(dma_sem, 16)
            sync.wait_ge(dve_sem, 5 * T + 2)  # incl. PSUM->SBUF copies
            sync.wait_ge(act_sem, 4 * T)
            sync.dma_start(
                out=dout[:, : 5 * NCHUNK], in_=acc_d[:]
            ).then_inc(dma_sem, 16)
            sync.dma_start(
                out=dout[:, 5 * NCHUNK :], in_=acc_a[:]
            ).then_inc(dma_sem, 16)
            sync.dma_start(out=dout2[:], in_=csum[:]).then_inc(dma_sem, 16)
            sync.wait_ge(dma_sem, 64 * T + 48)

        @block.gpsimd
        def _(gpsimd):
            for jj in range(T):
                bi = jj % NB
                gpsimd.wait_ge(dma_sem, 64 * (jj + 1))
                if jj >= NB:
                    # tA/tB/tC[bi] readers from chunk jj-2
                    gpsimd.wait_ge(dve_sem, 5 * (jj - 1))  # stt2, reluc
                    gpsimd.wait_ge(act_sem, 4 * (jj - 2) + 1)  # absA read tA
                nc.gpsimd.tensor_sub(
                    out=tA[bi][:], in0=tP[bi][:], in1=tT[bi][:]
                ).then_inc(pool_sem, 1)
                nc.gpsimd.tensor_sub(
                    out=tB[bi][:], in0=tTP[bi][:], in1=tTT[bi][:]
                ).then_inc(pool_sem, 1)
                nc.gpsimd.tensor_sub(
                    out=tC[bi][:], in0=tT[bi][:], in1=tTT[bi][:]
                ).then_inc(pool_sem, 1)

        @block.vector
        def _(vector):
            for jj in range(T):
                j = jj % NCHUNK
                bi = jj % NB
                vector.wait_ge(dma_sem, 64 * (jj + 1))
                if jj >= NB:
                    vector.wait_ge(act_sem, 4 * (jj - 2) + 3)  # e1 read tNP[bi]
                # nap = -|p| with free row-sum of -|p| (early: feeds ACT e1)
                nc.vector.scalar_tensor_tensor(
                    out=tNP[bi][:], in0=tP[bi][:], scalar=-1.0, in1=tP[bi][:],
                    op0=Alu.mult, op1=Alu.min, accum_out=dcol(j, 2),
                ).then_inc(dve_sem, 1)
                # S(-p*tp)
                nc.vector.scalar_tensor_tensor(
                    out=trash[:], in0=tP[bi][:], scalar=-1.0, in1=tTP[bi][:],
                    op0=Alu.mult, op1=Alu.mult, accum_out=dcol(j, 0),
                ).then_inc(dve_sem, 1)
                # S(-2500*a*b)
                vector.wait_ge(pool_sem, 3 * jj + 2)  # a, b ready
                nc.vector.scalar_tensor_tensor(
                    out=trash[:], in0=tA[bi][:], scalar=-2500.0, in1=tB[bi][:],
                    op0=Alu.mult, op1=Alu.mult, accum_out=dcol(j, 1),
                ).then_inc(dve_sem, 1)
                # S(p) (tensor_scalar cache-reduce: op1 is the reduce op)
                nc.vector.tensor_scalar(
                    out=trash[:], in0=tP[bi][:], scalar1=0.0, scalar2=0.0,
                    op0=Alu.add, op1=Alu.add, accum_out=dcol(j, 3),
                ).then_inc(dve_sem, 1)
                # S(relu(c))
                vector.wait_ge(pool_sem, 3 * jj + 3)  # c ready
                nc.vector.tensor_scalar(
                    out=trash[:], in0=tC[bi][:], scalar1=0.0, scalar2=0.0,
                    op0=Alu.max, op1=Alu.add, accum_out=dcol(j, 4),
                ).then_inc(dve_sem, 1)
            # PSUM -> SBUF staging
            vector.wait_ge(pe_sem, 8 * T)
            nc.vector.tensor_copy(
                out=csum[0:1, 0:512], in_=psum_t[0:1, :]
            ).then_inc(dve_sem, 1)
            nc.vector.tensor_copy(
                out=csum[0:1, 512:1024], in_=psum_tt[0:1, :]
            ).then_inc(dve_sem, 1)

        @block.scalar
        def _(scalar):
            for jj in range(T):
                j = jj % NCHUNK
                bi = jj % NB
                scalar.wait_ge(pool_sem, 3 * jj + 1)  # a ready
                nc.scalar.activation(
                    tAA[:], tA[bi][:], Act.Abs, accum_out=acol(j, 0)
                ).then_inc(act_sem, 1)
                nc.scalar.activation(
                    tE[:, F : 2 * F], tAA[:], Act.Exp, scale=-R
                ).then_inc(act_sem, 1)
                scalar.wait_ge(dve_sem, 5 * jj + 1)  # nap ready
                nc.scalar.activation(
                    tE[:, 0:F], tNP[bi][:], Act.Exp
                ).then_inc(act_sem, 1)
                nc.scalar.activation(
                    tF[:], tE[:], Act.Ln, bias=1.0, accum_out=acol(j, 1)
                ).then_inc(act_sem, 1)

        @block.tensor
        def _(tensor):
            for jj in range(T):
                bi = jj % NB
                tensor.wait_ge(dma_sem, 64 * (jj + 1))
                for k, (lo, hi) in enumerate(KSL):
                    w = hi - lo
                    nc.tensor.matmul(
                        psum_t[0:1, 0:w],
                        ones,
                        tT[bi][:, lo:hi],
                        start=(jj == 0 and k == 0),
                        stop=(jj == T - 1 and k == 3),
                    ).then_inc(pe_sem, 1)
                for k, (lo, hi) in enumerate(KSL):
                    w = hi - lo
                    nc.tensor.matmul(
                        psum_tt[0:1, 0:w],
                        ones,
                        tTT[bi][:, lo:hi],
                        start=(jj == 0 and k == 0),
                        stop=(jj == T - 1 and k == 3),
                    ).then_inc(pe_sem, 1)

    _CACHE[nloop] = (nc, bass_utils)
    return _CACHE[nloop]


def _run_device(shards, **kwargs):
    """shards: dict name -> list of 8 [P, FDIM] f32 arrays."""
    nc, bass_utils = _build()
    in_maps = [
        {name: shards[name][c] for name in ("p", "t", "tp", "tt")}
        for c in range(N_CORES)
    ]
    return bass_utils.run_bass_kernel_spmd(
        nc, in_maps, core_ids=list(range(N_CORES)), **kwargs
    )


def _shard(arr):
    flat = np.ascontiguousarray(arr, dtype=np.float32).reshape(-1)
    return [
        flat[c * PER_CORE : (c + 1) * PER_CORE].reshape(P, FDIM)
        for c in range(N_CORES)
    ]


def _reduce_host(results):
    # acc_out: [0:20] DVE chunk-major (stt1=S(-p*tp), stt2=S(-2500ab),
    # nap=S(-|p|), sump=S(p), reluc=S(relu(c))), [20:28] ACT chunk-major
    # (absA=S(|a|), lnC=S(ln1p_p)+S(ln1p_a)).
    # colsum_out: [0:512] S(t) cols, [512:1024] S(tt) cols.
    #   S(sp(p))   = 0.5 S(p) + 0.5 S(|p|) + lnC_p
    #   S(sp(50a)) = 25 S(p) - 25 S(t) + 25 S(|a|) + lnC_a
    #   10 S(|c|)  = 20 S(relu(c)) - 10 S(t) + 10 S(tt)
    cd = np.array([1.0, 1.0, -0.5, 0.5 + R / 2.0, 2.0 * BETA])
    ca = np.array([R / 2.0, 1.0])  # absA, lnC
    total = 0.0
    for c in range(N_CORES):
        out = results[c]["acc_out"].astype(np.float64)
        dve = out[:, : 5 * NCHUNK].reshape(P, NCHUNK, 5)
        act = out[:, 5 * NCHUNK :].reshape(P, NCHUNK, 2)
        total += float((dve.sum(axis=(0, 1)) * cd).sum())
        total += float((act.sum(axis=(0, 1)) * ca).sum())
        cs = results[c]["colsum_out"].astype(np.float64).reshape(1024)
        total += -(R / 2.0 + BETA) * cs[0:512].sum()  # S(t)
        total += BETA * cs[512:1024].sum()  # S(tt)
    return np.float32(total / NTOT)


def _numpy_fallback(p, t, tp, tt):
    """Exact reference semantics in float32 numpy (only used if the top-k
    collapse precondition ever fails)."""

    def bce(x, tgt):
        return (
            np.maximum(x, 0.0) - x * tgt + np.log1p(np.exp(-np.abs(x)))
        ).astype(np.float32)

    def balanced(x, tgt):
        losses = bce(x, tgt).ravel()
        mask = tgt.ravel() > 0.5
        n_pos = int(mask.sum())
        n_neg_avail = mask.size - n_pos
        n_negative = min(n_neg_avail, K * n_pos)
        pos_sum = np.float32(losses[mask].sum())
        neg_sorted = np.sort(losses[~mask])[::-1]
        neg_sum = np.float32(neg_sorted[:n_negative].sum())
        return (pos_sum + neg_sum) / np.float32(n_pos + n_negative)

    bin_map = (R * (p - t)).astype(np.float32)
    target_bin = (R * (tp - tt)).astype(np.float32)
    ls = balanced(p, tp)
    lb = balanced(bin_map, target_bin)
    lt = np.abs(t - tt).mean(dtype=np.float32)
    return np.float32(ls + ALPHA * lb + BETA * lt)


def kernel(
    proba_map, thresh_map, target_proba_map, target_thresh_map
) -> np.ndarray:
    p = np.asarray(proba_map, dtype=np.float32)
    t = np.asarray(thresh_map, dtype=np.float32)
    tp = np.asarray(target_proba_map, dtype=np.float32)
    tt = np.asarray(target_thresh_map, dtype=np.float32)

    # The device kernel assumes the hard-negative top-k keeps every negative
    # (n_neg_avail <= K*n_pos for both BCE terms). Cheap host check; exact
    # fallback otherwise.
    npos1 = int(np.count_nonzero(tp > 0.5))
    d = (R * (tp - tt)).astype(np.float32)
    npos2 = int(np.count_nonzero(d > 0.5))
    if (tp.size - npos1) > K * npos1 or (d.size - npos2) > K * npos2:
        return _numpy_fallback(p, t, tp, tt)

    shards = {"p": _shard(p), "t": _shard(t), "tp": _shard(tp), "tt": _shard(tt)}
    res = _run_device(shards)
    return _reduce_host(res.results)


# revision 13
# speedup vs baseline: 1.2019x; 1.2019x over previous
"""Trainium2 Bass kernel for nn_DBLoss_11605001634022.

DBLoss = Ls + Lb + 10*Lt over four (16,640,640) f32 maps, where Ls/Lb are
"balanced" BCE-with-logits losses with hard-negative mining (keep the top
n_negative = min(n_neg_avail, 3*n_pos) negative losses) and
Lt = mean|thresh - target_thresh|.

For these inputs the targets are ~uniform, so n_neg_avail <= 3*n_pos by a
huge margin and the top-k keeps ALL negatives; each balanced BCE collapses
to a plain mean of the elementwise BCE losses. With
bce(x, t) = softplus(x) - x*t, the whole loss is one streaming reduction:

  loss = [ S(sp(p)) - S(p*tp) + S(sp(50*a)) - 2500*S(a*b) + 10*S(|c|) ] / N
  a = p - t,  b = tp - tt,  c = t - tt,   S = sum over all elements

The kernel verifies the collapse condition on the host (cheap) and falls
back to an exact numpy implementation if it ever fails.

The HW has no softplus ACT table, so softplus uses the relu identity
  S(sp(x)) = (S(x) + S(|x|))/2 + S(ln(1 + exp(-|x|)))
with exp/ln in the single `natural_log_exp_and_others` ACT table set (one
table load, no switches). Likewise S(|c|) = 2 S(max(t,tt)) - S(tt)... - S(t) (relu identities).

Sharded batch-parallel: 2 images/core across 8 cores; each core streams
its 13.1 MB once, in 4 double-buffered [128,1600] chunks, every engine
loaded at/below the ~7.7 us/chunk DMA cadence (raw Bass + manual
semaphores; the Tile layer's multi-wait sync is rejected by this walrus):
  DVE  (6 scalar_tensor_tensor ops, each with a free row-sum):
        a=(p*1)-t (+S(a)); nap=(p*-1) min p = -|p| (+S(-|p|));
        (p*-1)*tp (+S); (a*-2500)*tp and (a*2500)*tt (+S, the expanded
        -2500*a*b term); (t*1) max tt (+S(max), for the relu identity
        S(relu(t-tt)) = S(max(t,tt)) - S(tt)).
  ACT  (4 passes): |a| (+row-sum); exp(-50|a|) and exp(-|p|) into one
        [128,3200] buffer; single merged ln(1+u) pass (+row-sum).
  PE   (8 fp32 ones-matmuls): column sums of t and tt accumulated in
        PSUM across chunks (exact, PE idle otherwise).
Row-sums land in per-engine stats tiles (no cross-engine SBUF write
granule sharing); PSUM column sums staged to SBUF at the end. Host applies
coefficients and the final division in float64.
"""

import numpy as np

N_CORES = 8
SHAPE = (16, 640, 640)
NTOT = SHAPE[0] * SHAPE[1] * SHAPE[2]
PER_CORE = NTOT // N_CORES  # 819200
P = 128
FDIM = PER_CORE // P  # 6400
NCHUNK = 4
F = FDIM // NCHUNK  # 1600
R = 50.0
ALPHA = 1.0
BETA = 10.0
K = 3

_CACHE = {}


def _get_concourse():
    try:
        import concourse.bass  # noqa: F401
    except ImportError:
        import sys

        sys.path.insert(0, "/opt/trn_rl_repo")
    import concourse.bass as bass
    import concourse.mybir as mybir
    from concourse import bass_utils

    return bass, mybir, bass_utils


def _build(nloop=1):
    """Build the bass program. nloop > 1 repeats the whole pipeline nloop
    times inside one NEFF (same result; used for dispatch-free timing)."""
    if nloop in _CACHE:
        return _CACHE[nloop]
    import contextlib

    bass, mybir, bass_utils = _get_concourse()
    f32 = mybir.dt.float32
    Alu = mybir.AluOpType
    Act = mybir.ActivationFunctionType

    nc = bass.Bass()
    dp = nc.dram_tensor("p", [P, FDIM], f32, kind="ExternalInput")
    dt_ = nc.dram_tensor("t", [P, FDIM], f32, kind="ExternalInput")
    dtp = nc.dram_tensor("tp", [P, FDIM], f32, kind="ExternalInput")
    dtt = nc.dram_tensor("tt", [P, FDIM], f32, kind="ExternalInput")
    dout = nc.dram_tensor("acc_out", [P, 8 * NCHUNK], f32, kind="ExternalOutput")
    dout2 = nc.dram_tensor("colsum_out", [1, 1024], f32, kind="ExternalOutput")

    NB = 2
    KSL = [(0, 512), (512, 1024), (1024, 1536), (1536, 1600)]
    T = nloop * NCHUNK

    ctx = contextlib.ExitStack()
    with ctx:
        sb = lambda name, shape: ctx.enter_context(
            nc.sbuf_tensor(name, shape, f32)
        )
        tP = [sb(f"tP{i}", [P, F]) for i in range(NB)]
        tT = [sb(f"tT{i}", [P, F]) for i in range(NB)]
        tTP = [sb(f"tTP{i}", [P, F]) for i in range(NB)]
        tTT = [sb(f"tTT{i}", [P, F]) for i in range(NB)]
        tA = [sb(f"tA{i}", [P, F]) for i in range(NB)]
        tNP = [sb(f"tNP{i}", [P, F]) for i in range(NB)]
        tAA = sb("tAA", [P, F])
        tE = sb("tE", [P, 2 * F])  # exp outputs (p-half | a-half)
        tF = sb("tF", [P, 2 * F])  # ln dump
        trash = sb("trash", [P, F])
        acc_d = sb("acc_d", [P, 6 * NCHUNK])
        acc_a = sb("acc_a", [P, 2 * NCHUNK])  # absA, lnC per chunk
        csum = sb("csum", [1, 1024])
        psum_t = ctx.enter_context(nc.psum_tensor("psum_t", [1, 512], f32))
        psum_tt = ctx.enter_context(nc.psum_tensor("psum_tt", [1, 512], f32))

        ones = nc.const_aps.tensor(1.0, (P, 1), f32)

        dma_sem = ctx.enter_context(nc.semaphore())
        dve_sem = ctx.enter_context(nc.semaphore())
        act_sem = ctx.enter_context(nc.semaphore())
        pe_sem = ctx.enter_context(nc.semaphore())
        block = ctx.enter_context(nc.Block())

        def dcol(j, k):
            return acc_d[:, 6 * j + k : 6 * j + k + 1]

        def acol(j, k):
            return acc_a[:, 2 * j + k : 2 * j + k + 1]

        @block.sync
        def _(sync):
            for jj in range(T):
                j = jj % NCHUNK
                bi = jj % NB
                sl = slice(j * F, (j + 1) * F)
                if jj >= NB:
                    # input buffers of chunk jj-2 must be fully consumed
                    sync.wait_ge(dve_sem, 6 * (jj - 1))
                    sync.wait_ge(pe_sem, 8 * (jj - 1))
                sync.dma_start(out=tP[bi][:], in_=dp[:, sl]).then_inc(dma_sem, 16)
                sync.dma_start(out=tT[bi][:], in_=dt_[:, sl]).then_inc(dma_sem, 16)
                sync.dma_start(out=tTP[bi][:], in_=dtp[:, sl]).then_inc(dma_sem, 16)
                sync.dma_start(out=tTT[bi][:], in_=dtt[:, sl]).then_inc(dma_sem, 16)
            sync.wait_ge(dve_sem, 6 * T + 2)  # incl. PSUM->SBUF copies
            sync.wait_ge(act_sem, 4 * T)
            sync.dma_start(
                out=dout[:, : 6 * NCHUNK], in_=acc_d[:]
            ).then_inc(dma_sem, 16)
            sync.dma_start(
                out=dout[:, 6 * NCHUNK :], in_=acc_a[:]
            ).then_inc(dma_sem, 16)
            sync.dma_start(out=dout2[:], in_=csum[:]).then_inc(dma_sem, 16)
            sync.wait_ge(dma_sem, 64 * T + 48)

        @block.vector
        def _(vector):
            for jj in range(T):
                j = jj % NCHUNK
                bi = jj % NB
                vector.wait_ge(dma_sem, 64 * (jj + 1))
                if jj >= NB:
                    vector.wait_ge(act_sem, 4 * (jj - 2) + 1)  # absA read tA
                # a = p - t, with free S(a)
                nc.vector.scalar_tensor_tensor(
                    out=tA[bi][:], in0=tP[bi][:], scalar=1.0, in1=tT[bi][:],
                    op0=Alu.mult, op1=Alu.subtract, accum_out=dcol(j, 3),
                ).then_inc(dve_sem, 1)
                if jj >= NB:
                    vector.wait_ge(act_sem, 4 * (jj - 2) + 3)  # e1 read tNP
                # nap = -|p|, with free S(-|p|)
                nc.vector.scalar_tensor_tensor(
                    out=tNP[bi][:], in0=tP[bi][:], scalar=-1.0, in1=tP[bi][:],
                    op0=Alu.mult, op1=Alu.min, accum_out=dcol(j, 2),
                ).then_inc(dve_sem, 1)
                # S(-p*tp)
                nc.vector.scalar_tensor_tensor(
                    out=trash[:], in0=tP[bi][:], scalar=-1.0, in1=tTP[bi][:],
                    op0=Alu.mult, op1=Alu.mult, accum_out=dcol(j, 0),
                ).then_inc(dve_sem, 1)
                # -2500*S(a*b) expanded: S(-2500*a*tp) + S(2500*a*tt)
                nc.vector.scalar_tensor_tensor(
                    out=trash[:], in0=tA[bi][:], scalar=-2500.0, in1=tTP[bi][:],
                    op0=Alu.mult, op1=Alu.mult, accum_out=dcol(j, 1),
                ).then_inc(dve_sem, 1)
                nc.vector.scalar_tensor_tensor(
                    out=trash[:], in0=tA[bi][:], scalar=2500.0, in1=tTT[bi][:],
                    op0=Alu.mult, op1=Alu.mult, accum_out=dcol(j, 4),
                ).then_inc(dve_sem, 1)
                # S(max(t,tt)): S(relu(t-tt)) = S(max) - S(tt)
                nc.vector.scalar_tensor_tensor(
                    out=trash[:], in0=tT[bi][:], scalar=1.0, in1=tTT[bi][:],
                    op0=Alu.mult, op1=Alu.max, accum_out=dcol(j, 5),
                ).then_inc(dve_sem, 1)
            # PSUM -> SBUF staging
            vector.wait_ge(pe_sem, 8 * T)
            nc.vector.tensor_copy(
                out=csum[0:1, 0:512], in_=psum_t[0:1, :]
            ).then_inc(dve_sem, 1)
            nc.vector.tensor_copy(
                out=csum[0:1, 512:1024], in_=psum_tt[0:1, :]
            ).then_inc(dve_sem, 1)

        @block.scalar
        def _(scalar):
            for jj in range(T):
                j = jj % NCHUNK
                bi = jj % NB
                scalar.wait_ge(dve_sem, 6 * jj + 1)  # a ready
                nc.scalar.activation(
                    tAA[:], tA[bi][:], Act.Abs, accum_out=acol(j, 0)
                ).then_inc(act_sem, 1)
                nc.scalar.activation(
                    tE[:, F : 2 * F], tAA[:], Act.Exp, scale=-R
                ).then_inc(act_sem, 1)
                scalar.wait_ge(dve_sem, 6 * jj + 2)  # nap ready
                nc.scalar.activation(
                    tE[:, 0:F], tNP[bi][:], Act.Exp
                ).then_inc(act_sem, 1)
                nc.scalar.activation(
                    tF[:], tE[:], Act.Ln, bias=1.0, accum_out=acol(j, 1)
                ).then_inc(act_sem, 1)

        @block.tensor
        def _(tensor):
            for jj in range(T):
                bi = jj % NB
                tensor.wait_ge(dma_sem, 64 * (jj + 1))
                for k, (lo, hi) in enumerate(KSL):
                    w = hi - lo
                    nc.tensor.matmul(
                        psum_t[0:1, 0:w],
                        ones,
                        tT[bi][:, lo:hi],
                        start=(jj % NCHUNK == 0 and k == 0),
                        stop=(jj % NCHUNK == NCHUNK - 1 and k == 3),
                    ).then_inc(pe_sem, 1)
                for k, (lo, hi) in enumerate(KSL):
                    w = hi - lo
                    nc.tensor.matmul(
                        psum_tt[0:1, 0:w],
                        ones,
                        tTT[bi][:, lo:hi],
                        start=(jj % NCHUNK == 0 and k == 0),
                        stop=(jj % NCHUNK == NCHUNK - 1 and k == 3),
                    ).then_inc(pe_sem, 1)

    _CACHE[nloop] = (nc, bass_utils)
    return _CACHE[nloop]


def _run_device(shards, **kwargs):
    """shards: dict name -> list of 8 [P, FDIM] f32 arrays."""
    nc, bass_utils = _build()
    in_maps = [
        {name: shards[name][c] for name in ("p", "t", "tp", "tt")}
        for c in range(N_CORES)
    ]
    return bass_utils.run_bass_kernel_spmd(
        nc, in_maps, core_ids=list(range(N_CORES)), **kwargs
    )


def _shard(arr):
    flat = np.ascontiguousarray(arr, dtype=np.float32).reshape(-1)
    return [
        flat[c * PER_CORE : (c + 1) * PER_CORE].reshape(P, FDIM)
        for c in range(N_CORES)
    ]


def _reduce_host(results):
    # acc_out: [0:24] DVE chunk-major (stt1=S(-p*tp), stt2a=S(-2500*a*tp),
    # nap=S(-|p|), suma=S(a), stt2b=S(2500*a*tt), smax=S(max(t,tt))),
    # [24:32] ACT chunk-major (absA=S(|a|), lnC=S(ln1p_p)+S(ln1p_a)).
    # colsum_out: [0:512] S(t) cols, [512:1024] S(tt) cols.
    #   S(sp(p))   = 0.5 (S(a)+S(t)) + 0.5 S(|p|) + lnC_p
    #   S(sp(50a)) = 25 S(a) + 25 S(|a|) + lnC_a
    #   10 S(|c|)  = 20 S(max(t,tt)) - 10 S(tt) - 10 S(t)
    cd = np.array([1.0, 1.0, -0.5, 0.5 + R / 2.0, 1.0, 2.0 * BETA])
    ca = np.array([R / 2.0, 1.0])  # absA, lnC
    total = 0.0
    for c in range(N_CORES):
        out = results[c]["acc_out"].astype(np.float64)
        dve = out[:, : 6 * NCHUNK].reshape(P, NCHUNK, 6)
        act = out[:, 6 * NCHUNK :].reshape(P, NCHUNK, 2)
        total += float((dve.sum(axis=(0, 1)) * cd).sum())
        total += float((act.sum(axis=(0, 1)) * ca).sum())
        cs = results[c]["colsum_out"].astype(np.float64).reshape(1024)
        total += (0.5 - BETA) * cs[0:512].sum()  # S(t)
        total += -BETA * cs[512:1024].sum()  # S(tt)
    return np.float32(total / NTOT)


def _numpy_fallback(p, t, tp, tt):
    """Exact reference semantics in float32 numpy (only used if the top-k
    collapse precondition ever fails)."""

    def bce(x, tgt):
        return (
            np.maximum(x, 0.0) - x * tgt + np.log1p(np.exp(-np.abs(x)))
        ).astype(np.float32)

    def balanced(x, tgt):
        losses = bce(x, tgt).ravel()
        mask = tgt.ravel() > 0.5
        n_pos = int(mask.sum())
        n_neg_avail = mask.size - n_pos
        n_negative = min(n_neg_avail, K * n_pos)
        pos_sum = np.float32(losses[mask].sum())
        neg_sorted = np.sort(losses[~mask])[::-1]
        neg_sum = np.float32(neg_sorted[:n_negative].sum())
        return (pos_sum + neg_sum) / np.float32(n_pos + n_negative)

    bin_map = (R * (p - t)).astype(np.float32)
    target_bin = (R * (tp - tt)).astype(np.float32)
    ls = balanced(p, tp)
    lb = balanced(bin_map, target_bin)
    lt = np.abs(t - tt).mean(dtype=np.float32)
    return np.float32(ls + ALPHA * lb + BETA * lt)


def kernel(
    proba_map, thresh_map, target_proba_map, target_thresh_map
) -> np.ndarray:
    p = np.asarray(proba_map, dtype=np.float32)
    t = np.asarray(thresh_map, dtype=np.float32)
    tp = np.asarray(target_proba_map, dtype=np.float32)
    tt = np.asarray(target_thresh_map, dtype=np.float32)

    # The device kernel assumes the hard-negative top-k keeps every negative
    # (n_neg_avail <= K*n_pos for both BCE terms). Cheap host check; exact
    # fallback otherwise.
    npos1 = int(np.count_nonzero(tp > 0.5))
    d = (R * (tp - tt)).astype(np.float32)
    npos2 = int(np.count_nonzero(d > 0.5))
    if (tp.size - npos1) > K * npos1 or (d.size - npos2) > K * npos2:
        return _numpy_fallback(p, t, tp, tt)

    shards = {"p": _shard(p), "t": _shard(t), "tp": _shard(tp), "tt": _shard(tt)}
    res = _run_device(shards)
    return _reduce_host(res.results)


# revision 14
# speedup vs baseline: 1.3081x; 1.0884x over previous
"""Trainium2 Bass kernel for nn_DBLoss_11605001634022.

DBLoss = Ls + Lb + 10*Lt over four (16,640,640) f32 maps, where Ls/Lb are
"balanced" BCE-with-logits losses with hard-negative mining (keep the top
n_negative = min(n_neg_avail, 3*n_pos) negative losses) and
Lt = mean|thresh - target_thresh|.

For these inputs the targets are ~uniform, so n_neg_avail <= 3*n_pos by a
huge margin and the top-k keeps ALL negatives; each balanced BCE collapses
to a plain mean of the elementwise BCE losses. With
bce(x, t) = softplus(x) - x*t, the whole loss is one streaming reduction:

  loss = [ S(sp(p)) - S(p*tp) + S(sp(50*a)) - 2500*S(a*b) + 10*S(|c|) ] / N
  a = p - t,  b = tp - tt,  c = t - tt,   S = sum over all elements

The kernel verifies the collapse condition on the host (cheap) and falls
back to an exact numpy implementation if it ever fails.

The HW has no softplus ACT table, so softplus uses the relu identity
  S(sp(x)) = (S(x) + S(|x|))/2 + S(ln(1 + exp(-|x|)))
with exp/ln in the single `natural_log_exp_and_others` ACT table set (one
table load, no switches). Likewise S(|c|) = 2 S(max(t,tt)) - S(tt)... - S(t) (relu identities).

Sharded batch-parallel: 2 images/core across 8 cores; each core streams
its 13.1 MB once, in 4 double-buffered [128,1600] chunks, every engine
loaded at/below the ~7.7 us/chunk DMA cadence (raw Bass + manual
semaphores; the Tile layer's multi-wait sync is rejected by this walrus):
  DVE  (6 scalar_tensor_tensor ops, each with a free row-sum):
        a=(p*1)-t (+S(a)); nap=(p*-1) min p = -|p| (+S(-|p|));
        (p*-1)*tp (+S); (a*-2500)*tp and (a*2500)*tt (+S, the expanded
        -2500*a*b term); (t*1) max tt (+S(max), for the relu identity
        S(relu(t-tt)) = S(max(t,tt)) - S(tt)).
  ACT  (4 passes): |a| (+row-sum); exp(-50|a|) and exp(-|p|) into one
        [128,3200] buffer; single merged ln(1+u) pass (+row-sum).
  PE   (8 fp32 ones-matmuls): column sums of t and tt accumulated in
        PSUM across chunks (exact, PE idle otherwise).
Row-sums land in per-engine stats tiles (no cross-engine SBUF write
granule sharing); PSUM column sums staged to SBUF at the end. Host applies
coefficients and the final division in float64.
"""

import numpy as np

N_CORES = 8
SHAPE = (16, 640, 640)
NTOT = SHAPE[0] * SHAPE[1] * SHAPE[2]
PER_CORE = NTOT // N_CORES  # 819200
P = 128
FDIM = PER_CORE // P  # 6400
NCHUNK = 4
F = FDIM // NCHUNK  # 1600
R = 50.0
ALPHA = 1.0
BETA = 10.0
K = 3

_CACHE = {}


def _get_concourse():
    try:
        import concourse.bass  # noqa: F401
    except ImportError:
        import sys

        sys.path.insert(0, "/opt/trn_rl_repo")
    import concourse.bass as bass
    import concourse.mybir as mybir
    from concourse import bass_utils

    return bass, mybir, bass_utils


def _build(nloop=1):
    """Build the bass program. nloop > 1 repeats the whole pipeline nloop
    times inside one NEFF (same result; used for dispatch-free timing)."""
    if nloop in _CACHE:
        return _CACHE[nloop]
    import contextlib

    bass, mybir, bass_utils = _get_concourse()
    f32 = mybir.dt.float32
    Alu = mybir.AluOpType
    Act = mybir.ActivationFunctionType

    nc = bass.Bass()
    dp = nc.dram_tensor("p", [P, FDIM], f32, kind="ExternalInput")
    dt_ = nc.dram_tensor("t", [P, FDIM], f32, kind="ExternalInput")
    dtp = nc.dram_tensor("tp", [P, FDIM], f32, kind="ExternalInput")
    dtt = nc.dram_tensor("tt", [P, FDIM], f32, kind="ExternalInput")
    dout = nc.dram_tensor("acc_out", [P, 8 * NCHUNK], f32, kind="ExternalOutput")
    dout2 = nc.dram_tensor("colsum_out", [1, 1024], f32, kind="ExternalOutput")

    NB = 2  # intermediate (tA/tNP) buffers
    NBI = 3  # input tile buffers
    KSL = [(0, 512), (512, 1024), (1024, 1536), (1536, 1600)]
    T = nloop * NCHUNK

    ctx = contextlib.ExitStack()
    with ctx:
        sb = lambda name, shape: ctx.enter_context(
            nc.sbuf_tensor(name, shape, f32)
        )
        tP = [sb(f"tP{i}", [P, F]) for i in range(NBI)]
        tT = [sb(f"tT{i}", [P, F]) for i in range(NBI)]
        tTP = [sb(f"tTP{i}", [P, F]) for i in range(NBI)]
        tTT = [sb(f"tTT{i}", [P, F]) for i in range(NBI)]
        tA = [sb(f"tA{i}", [P, F]) for i in range(NB)]
        tNP = [sb(f"tNP{i}", [P, F]) for i in range(NB)]
        tAA = sb("tAA", [P, F])
        tE = sb("tE", [P, 2 * F])  # exp outputs (p-half | a-half)
        tF = sb("tF", [P, 2 * F])  # ln dump
        trash = sb("trash", [P, F])
        acc_d = sb("acc_d", [P, 6 * NCHUNK])
        acc_a = sb("acc_a", [P, 2 * NCHUNK])  # absA, lnC per chunk
        csum = sb("csum", [1, 1024])
        psum_t = ctx.enter_context(nc.psum_tensor("psum_t", [1, 512], f32))
        psum_tt = ctx.enter_context(nc.psum_tensor("psum_tt", [1, 512], f32))

        ones = nc.const_aps.tensor(1.0, (P, 1), f32)

        dma_sem = ctx.enter_context(nc.semaphore())
        dve_sem = ctx.enter_context(nc.semaphore())
        act_sem = ctx.enter_context(nc.semaphore())
        pe_sem = ctx.enter_context(nc.semaphore())
        block = ctx.enter_context(nc.Block())

        def dcol(j, k):
            return acc_d[:, 6 * j + k : 6 * j + k + 1]

        def acol(j, k):
            return acc_a[:, 2 * j + k : 2 * j + k + 1]

        @block.sync
        def _(sync):
            for jj in range(T):
                j = jj % NCHUNK
                bi = jj % NBI
                sl = slice(j * F, (j + 1) * F)
                if jj >= NBI:
                    # input buffers of chunk jj-3 must be fully consumed
                    sync.wait_ge(dve_sem, 6 * (jj - 2))
                    sync.wait_ge(pe_sem, 8 * (jj - 2))
                sync.dma_start(out=tP[bi][:], in_=dp[:, sl]).then_inc(dma_sem, 16)
                sync.dma_start(out=tT[bi][:], in_=dt_[:, sl]).then_inc(dma_sem, 16)
                sync.dma_start(out=tTP[bi][:], in_=dtp[:, sl]).then_inc(dma_sem, 16)
                sync.dma_start(out=tTT[bi][:], in_=dtt[:, sl]).then_inc(dma_sem, 16)
            sync.wait_ge(dve_sem, 6 * T + 2)  # incl. PSUM->SBUF copies
            sync.wait_ge(act_sem, 4 * T)
            sync.dma_start(
                out=dout[:, : 6 * NCHUNK], in_=acc_d[:]
            ).then_inc(dma_sem, 16)
            sync.dma_start(
                out=dout[:, 6 * NCHUNK :], in_=acc_a[:]
            ).then_inc(dma_sem, 16)
            sync.dma_start(out=dout2[:], in_=csum[:]).then_inc(dma_sem, 16)
            sync.wait_ge(dma_sem, 64 * T + 48)

        @block.vector
        def _(vector):
            for jj in range(T):
                j = jj % NCHUNK
                bi = jj % NB
                bii = jj % NBI
                vector.wait_ge(dma_sem, 64 * (jj + 1))
                if jj >= NB:
                    vector.wait_ge(act_sem, 4 * (jj - 2) + 1)  # absA read tA
                # a = p - t, with free S(a)
                nc.vector.scalar_tensor_tensor(
                    out=tA[bi][:], in0=tP[bii][:], scalar=1.0, in1=tT[bii][:],
                    op0=Alu.mult, op1=Alu.subtract, accum_out=dcol(j, 3),
                ).then_inc(dve_sem, 1)
                if jj >= NB:
                    vector.wait_ge(act_sem, 4 * (jj - 2) + 3)  # e1 read tNP
                # nap = -|p|, with free S(-|p|)
                nc.vector.scalar_tensor_tensor(
                    out=tNP[bi][:], in0=tP[bii][:], scalar=-1.0, in1=tP[bii][:],
                    op0=Alu.mult, op1=Alu.min, accum_out=dcol(j, 2),
                ).then_inc(dve_sem, 1)
                # S(-p*tp)
                nc.vector.scalar_tensor_tensor(
                    out=trash[:], in0=tP[bii][:], scalar=-1.0, in1=tTP[bii][:],
                    op0=Alu.mult, op1=Alu.mult, accum_out=dcol(j, 0),
                ).then_inc(dve_sem, 1)
                # -2500*S(a*b) expanded: S(-2500*a*tp) + S(2500*a*tt)
                nc.vector.scalar_tensor_tensor(
                    out=trash[:], in0=tA[bi][:], scalar=-2500.0, in1=tTP[bii][:],
                    op0=Alu.mult, op1=Alu.mult, accum_out=dcol(j, 1),
                ).then_inc(dve_sem, 1)
                nc.vector.scalar_tensor_tensor(
                    out=trash[:], in0=tA[bi][:], scalar=2500.0, in1=tTT[bii][:],
                    op0=Alu.mult, op1=Alu.mult, accum_out=dcol(j, 4),
                ).then_inc(dve_sem, 1)
                # S(max(t,tt)): S(relu(t-tt)) = S(max) - S(tt)
                nc.vector.scalar_tensor_tensor(
                    out=trash[:], in0=tT[bii][:], scalar=1.0, in1=tTT[bii][:],
                    op0=Alu.mult, op1=Alu.max, accum_out=dcol(j, 5),
                ).then_inc(dve_sem, 1)
            # PSUM -> SBUF staging
            vector.wait_ge(pe_sem, 8 * T)
            nc.vector.tensor_copy(
                out=csum[0:1, 0:512], in_=psum_t[0:1, :]
            ).then_inc(dve_sem, 1)
            nc.vector.tensor_copy(
                out=csum[0:1, 512:1024], in_=psum_tt[0:1, :]
            ).then_inc(dve_sem, 1)

        @block.scalar
        def _(scalar):
            for jj in range(T):
                j = jj % NCHUNK
                bi = jj % NB
                scalar.wait_ge(dve_sem, 6 * jj + 1)  # a ready
                nc.scalar.activation(
                    tAA[:], tA[bi][:], Act.Abs, accum_out=acol(j, 0)
                ).then_inc(act_sem, 1)
                nc.scalar.activation(
                    tE[:, F : 2 * F], tAA[:], Act.Exp, scale=-R
                ).then_inc(act_sem, 1)
                scalar.wait_ge(dve_sem, 6 * jj + 2)  # nap ready
                nc.scalar.activation(
                    tE[:, 0:F], tNP[bi][:], Act.Exp
                ).then_inc(act_sem, 1)
                nc.scalar.activation(
                    tF[:], tE[:], Act.Ln, bias=1.0, accum_out=acol(j, 1)
                ).then_inc(act_sem, 1)

        @block.tensor
        def _(tensor):
            for jj in range(T):
                bi = jj % NBI
                tensor.wait_ge(dma_sem, 64 * (jj + 1))
                for k, (lo, hi) in enumerate(KSL):
                    w = hi - lo
                    nc.tensor.matmul(
                        psum_t[0:1, 0:w],
                        ones,
                        tT[bi][:, lo:hi],
                        start=(jj % NCHUNK == 0 and k == 0),
                        stop=(jj % NCHUNK == NCHUNK - 1 and k == 3),
                    ).then_inc(pe_sem, 1)
                for k, (lo, hi) in enumerate(KSL):
                    w = hi - lo
                    nc.tensor.matmul(
                        psum_tt[0:1, 0:w],
                        ones,
                        tTT[bi][:, lo:hi],
                        start=(jj % NCHUNK == 0 and k == 0),
                        stop=(jj % NCHUNK == NCHUNK - 1 and k == 3),
                    ).then_inc(pe_sem, 1)

    _CACHE[nloop] = (nc, bass_utils)
    return _CACHE[nloop]


def _run_device(shards, **kwargs):
    """shards: dict name -> list of 8 [P, FDIM] f32 arrays."""
    nc, bass_utils = _build()
    in_maps = [
        {name: shards[name][c] for name in ("p", "t", "tp", "tt")}
        for c in range(N_CORES)
    ]
    return bass_utils.run_bass_kernel_spmd(
        nc, in_maps, core_ids=list(range(N_CORES)), **kwargs
    )


def _shard(arr):
    flat = np.ascontiguousarray(arr, dtype=np.float32).reshape(-1)
    return [
        flat[c * PER_CORE : (c + 1) * PER_CORE].reshape(P, FDIM)
        for c in range(N_CORES)
    ]


def _reduce_host(results):
    # acc_out: [0:24] DVE chunk-major (stt1=S(-p*tp), stt2a=S(-2500*a*tp),
    # nap=S(-|p|), suma=S(a), stt2b=S(2500*a*tt), smax=S(max(t,tt))),
    # [24:32] ACT chunk-major (absA=S(|a|), lnC=S(ln1p_p)+S(ln1p_a)).
    # colsum_out: [0:512] S(t) cols, [512:1024] S(tt) cols.
    #   S(sp(p))   = 0.5 (S(a)+S(t)) + 0.5 S(|p|) + lnC_p
    #   S(sp(50a)) = 25 S(a) + 25 S(|a|) + lnC_a
    #   10 S(|c|)  = 20 S(max(t,tt)) - 10 S(tt) - 10 S(t)
    cd = np.array([1.0, 1.0, -0.5, 0.5 + R / 2.0, 1.0, 2.0 * BETA])
    ca = np.array([R / 2.0, 1.0])  # absA, lnC
    total = 0.0
    for c in range(N_CORES):
        out = results[c]["acc_out"].astype(np.float64)
        dve = out[:, : 6 * NCHUNK].reshape(P, NCHUNK, 6)
        act = out[:, 6 * NCHUNK :].reshape(P, NCHUNK, 2)
        total += float((dve.sum(axis=(0, 1)) * cd).sum())
        total += float((act.sum(axis=(0, 1)) * ca).sum())
        cs = results[c]["colsum_out"].astype(np.float64).reshape(1024)
        total += (0.5 - BETA) * cs[0:512].sum()  # S(t)
        total += -BETA * cs[512:1024].sum()  # S(tt)
    return np.float32(total / NTOT)


def _numpy_fallback(p, t, tp, tt):
    """Exact reference semantics in float32 numpy (only used if the top-k
    collapse precondition ever fails)."""

    def bce(x, tgt):
        return (
            np.maximum(x, 0.0) - x * tgt + np.log1p(np.exp(-np.abs(x)))
        ).astype(np.float32)

    def balanced(x, tgt):
        losses = bce(x, tgt).ravel()
        mask = tgt.ravel() > 0.5
        n_pos = int(mask.sum())
        n_neg_avail = mask.size - n_pos
        n_negative = min(n_neg_avail, K * n_pos)
        pos_sum = np.float32(losses[mask].sum())
        neg_sorted = np.sort(losses[~mask])[::-1]
        neg_sum = np.float32(neg_sorted[:n_negative].sum())
        return (pos_sum + neg_sum) / np.float32(n_pos + n_negative)

    bin_map = (R * (p - t)).astype(np.float32)
    target_bin = (R * (tp - tt)).astype(np.float32)
    ls = balanced(p, tp)
    lb = balanced(bin_map, target_bin)
    lt = np.abs(t - tt).mean(dtype=np.float32)
    return np.float32(ls + ALPHA * lb + BETA * lt)


def kernel(
    proba_map, thresh_map, target_proba_map, target_thresh_map
) -> np.ndarray:
    p = np.asarray(proba_map, dtype=np.float32)
    t = np.asarray(thresh_map, dtype=np.float32)
    tp = np.asarray(target_proba_map, dtype=np.float32)
    tt = np.asarray(target_thresh_map, dtype=np.float32)

    # The device kernel assumes the hard-negative top-k keeps every negative
    # (n_neg_avail <= K*n_pos for both BCE terms). Cheap host check; exact
    # fallback otherwise.
    npos1 = int(np.count_nonzero(tp > 0.5))
    d = (R * (tp - tt)).astype(np.float32)
    npos2 = int(np.count_nonzero(d > 0.5))
    if (tp.size - npos1) > K * npos1 or (d.size - npos2) > K * npos2:
        return _numpy_fallback(p, t, tp, tt)

    shards = {"p": _shard(p), "t": _shard(t), "tp": _shard(tp), "tt": _shard(tt)}
    res = _run_device(shards)
    return _reduce_host(res.results)


# revision 20
# speedup vs baseline: 1.5023x; 1.1484x over previous
"""Trainium2 Bass kernel for nn_DBLoss_11605001634022.

DBLoss = Ls + Lb + 10*Lt over four (16,640,640) f32 maps, where Ls/Lb are
"balanced" BCE-with-logits losses with hard-negative mining (keep the top
n_negative = min(n_neg_avail, 3*n_pos) negative losses) and
Lt = mean|thresh - target_thresh|.

For these inputs the targets are ~uniform, so n_neg_avail <= 3*n_pos by a
huge margin and the top-k keeps ALL negatives; each balanced BCE collapses
to a plain mean of the elementwise BCE losses. With
bce(x, t) = softplus(x) - x*t, the whole loss is one streaming reduction:

  loss = [ S(sp(p)) - S(p*tp) + S(sp(50*a)) - 2500*S(a*b) + 10*S(|c|) ] / N
  a = p - t,  b = tp - tt,  c = t - tt,   S = sum over all elements

The kernel verifies the collapse condition on the host (cheap) and falls
back to an exact numpy implementation if it ever fails.

The HW has no softplus ACT table, so softplus uses the relu identity
  S(sp(x)) = (S(x) + S(|x|))/2 + S(ln(1 + exp(-|x|)))
with exp/ln in the single `natural_log_exp_and_others` ACT table set (one
table load, no switches). Likewise
  S(|c|) = 2 S(relu(c)) - S(c) = 2 S(max(t,tt)) - S(tt) - S(t).

Sharded batch-parallel: 2 images/core across 8 cores; each core streams
its 13.1 MB once, in 4 pipelined [128,1600] chunks (4-deep buffered input
and intermediate tiles), raw Bass + manual semaphores (the
Tile layer's multi-wait sync is rejected by this walrus). GPSIMD is kept
idle on purpose: its elementwise ops measured ~5x slower in situ than DVE
under full SBUF port contention, and DVE ops measured ~2.2 us each in
situ, so work is spread DVE/ACT/PE evenly. Per-tensor DMA semaphores let
each consumer start as soon as the specific tensor it needs has landed.
Per chunk:
  DVE  (5 scalar_tensor_tensor ops, each with a free row-sum):
        a=(p*1)-t (+S(a)); (p*-1)*tp (+S); (a*-2500)*tp and (a*2500)*tt
        (+S, the expanded -2500*a*b term); (t*1) max tt (+S(max), for the
        relu identity S(relu(t-tt)) = S(max(t,tt)) - S(tt)).
  ACT  (5 passes): |p| (+row-sum), exp(-|p|); |a| (+row-sum), exp(-50|a|)
        into one [128,3200] buffer; single merged ln(1+u) pass (+row-sum).
  PE   (8 fp32 ones-matmuls): column sums of t and tt accumulated in
        PSUM across chunks (exact, PE idle otherwise).
Row-sums land in per-engine stats tiles (no cross-engine SBUF write
granule sharing); PSUM column sums staged to SBUF at the end. Host applies
coefficients and the final division in float64.
"""

import numpy as np

N_CORES = 8
SHAPE = (16, 640, 640)
NTOT = SHAPE[0] * SHAPE[1] * SHAPE[2]
PER_CORE = NTOT // N_CORES  # 819200
P = 128
FDIM = PER_CORE // P  # 6400
NCHUNK = 4
F = FDIM // NCHUNK  # 1600
R = 50.0
ALPHA = 1.0
BETA = 10.0
K = 3

_CACHE = {}


def _get_concourse():
    try:
        import concourse.bass  # noqa: F401
    except ImportError:
        import sys

        sys.path.insert(0, "/opt/trn_rl_repo")
    import concourse.bass as bass
    import concourse.mybir as mybir
    from concourse import bass_utils

    return bass, mybir, bass_utils


def _build(nloop=1):
    """Build the bass program. nloop > 1 repeats the whole pipeline nloop
    times inside one NEFF (same result; used for dispatch-free timing)."""
    if nloop in _CACHE:
        return _CACHE[nloop]
    import contextlib

    bass, mybir, bass_utils = _get_concourse()
    f32 = mybir.dt.float32
    Alu = mybir.AluOpType
    Act = mybir.ActivationFunctionType

    nc = bass.Bass()
    dp = nc.dram_tensor("p", [P, FDIM], f32, kind="ExternalInput")
    dt_ = nc.dram_tensor("t", [P, FDIM], f32, kind="ExternalInput")
    dtp = nc.dram_tensor("tp", [P, FDIM], f32, kind="ExternalInput")
    dtt = nc.dram_tensor("tt", [P, FDIM], f32, kind="ExternalInput")
    dout = nc.dram_tensor("acc_out", [P, 8 * NCHUNK], f32, kind="ExternalOutput")
    dout2 = nc.dram_tensor("colsum_out", [1, 1024], f32, kind="ExternalOutput")

    NB = 4  # intermediate (tA) buffers
    NBI = 4  # input tile buffers
    KSL = [(0, 512), (512, 1024), (1024, 1536), (1536, 1600)]
    T = nloop * NCHUNK

    ctx = contextlib.ExitStack()
    with ctx:
        sb = lambda name, shape: ctx.enter_context(
            nc.sbuf_tensor(name, shape, f32)
        )
        tP = [sb(f"tP{i}", [P, F]) for i in range(NBI)]
        tT = [sb(f"tT{i}", [P, F]) for i in range(NBI)]
        tTP = [sb(f"tTP{i}", [P, F]) for i in range(NBI)]
        tTT = [sb(f"tTT{i}", [P, F]) for i in range(NBI)]
        tA = [sb(f"tA{i}", [P, F]) for i in range(NB)]
        tAP = sb("tAP", [P, F])
        tAA = sb("tAA", [P, F])
        tE = sb("tE", [P, 2 * F])  # exp outputs (p-half | a-half)
        tF = sb("tF", [P, 2 * F])  # ln dump
        trash = sb("trash", [P, F])
        acc_d = sb("acc_d", [P, 5 * NCHUNK])
        acc_a = sb("acc_a", [P, 3 * NCHUNK])  # absP, absA, lnC
        csum = sb("csum", [1, 1024])
        psum_t = ctx.enter_context(nc.psum_tensor("psum_t", [1, 512], f32))
        psum_tt = ctx.enter_context(nc.psum_tensor("psum_tt", [1, 512], f32))

        ones = nc.const_aps.tensor(1.0, (P, 1), f32)

        dma_p = ctx.enter_context(nc.semaphore())
        dma_t = ctx.enter_context(nc.semaphore())
        dma_tp = ctx.enter_context(nc.semaphore())
        dma_tt = ctx.enter_context(nc.semaphore())
        dve_sem = ctx.enter_context(nc.semaphore())
        act_sem = ctx.enter_context(nc.semaphore())
        pe_sem = ctx.enter_context(nc.semaphore())
        block = ctx.enter_context(nc.Block())

        def dcol(j, k):
            return acc_d[:, 5 * j + k : 5 * j + k + 1]

        def acol(j, k):
            return acc_a[:, 3 * j + k : 3 * j + k + 1]

        @block.sync
        def _(sync):
            for jj in range(T):
                j = jj % NCHUNK
                bi = jj % NBI
                sl = slice(j * F, (j + 1) * F)
                if jj >= NBI:
                    # input buffers of chunk jj-3 must be fully consumed
                    sync.wait_ge(dve_sem, 5 * (jj - 2))
                    sync.wait_ge(act_sem, 5 * (jj - 3) + 1)  # absP read tP
                    sync.wait_ge(pe_sem, 8 * (jj - 2))
                sync.dma_start(out=tP[bi][:], in_=dp[:, sl]).then_inc(dma_p, 16)
                sync.dma_start(out=tT[bi][:], in_=dt_[:, sl]).then_inc(dma_t, 16)
                sync.dma_start(out=tTP[bi][:], in_=dtp[:, sl]).then_inc(dma_tp, 16)
                sync.dma_start(out=tTT[bi][:], in_=dtt[:, sl]).then_inc(dma_tt, 16)
            sync.wait_ge(dve_sem, 5 * T + 2)  # incl. PSUM->SBUF copies
            sync.wait_ge(act_sem, 5 * T)
            sync.dma_start(
                out=dout[:, : 5 * NCHUNK], in_=acc_d[:]
            ).then_inc(dma_p, 16)
            sync.dma_start(
                out=dout[:, 5 * NCHUNK :], in_=acc_a[:]
            ).then_inc(dma_p, 16)
            sync.dma_start(out=dout2[:], in_=csum[:]).then_inc(dma_p, 16)
            sync.wait_ge(dma_p, 16 * T + 48)
            sync.wait_ge(dma_t, 16 * T)
            sync.wait_ge(dma_tp, 16 * T)
            sync.wait_ge(dma_tt, 16 * T)

        @block.vector
        def _(vector):
            for jj in range(T):
                j = jj % NCHUNK
                bi = jj % NB
                bii = jj % NBI
                vector.wait_ge(dma_p, 16 * (jj + 1))
                if jj >= NB:
                    # absA of chunk jj-3 must have read tA[bi]
                    vector.wait_ge(act_sem, 5 * (jj - NB) + 3)
                # a = p - t, with free S(a)
                vector.wait_ge(dma_t, 16 * (jj + 1))
                nc.vector.scalar_tensor_tensor(
                    out=tA[bi][:], in0=tP[bii][:], scalar=1.0, in1=tT[bii][:],
                    op0=Alu.mult, op1=Alu.subtract, accum_out=dcol(j, 3),
                ).then_inc(dve_sem, 1)
                # S(-p*tp)
                vector.wait_ge(dma_tp, 16 * (jj + 1))
                nc.vector.scalar_tensor_tensor(
                    out=trash[:], in0=tP[bii][:], scalar=-1.0, in1=tTP[bii][:],
                    op0=Alu.mult, op1=Alu.mult, accum_out=dcol(j, 0),
                ).then_inc(dve_sem, 1)
                # -2500*S(a*b) expanded: S(-2500*a*tp) + S(2500*a*tt)
                nc.vector.scalar_tensor_tensor(
                    out=trash[:], in0=tA[bi][:], scalar=-2500.0, in1=tTP[bii][:],
                    op0=Alu.mult, op1=Alu.mult, accum_out=dcol(j, 1),
                ).then_inc(dve_sem, 1)
                vector.wait_ge(dma_tt, 16 * (jj + 1))
                nc.vector.scalar_tensor_tensor(
                    out=trash[:], in0=tA[bi][:], scalar=2500.0, in1=tTT[bii][:],
                    op0=Alu.mult, op1=Alu.mult, accum_out=dcol(j, 2),
                ).then_inc(dve_sem, 1)
                # S(max(t,tt)): S(relu(t-tt)) = S(max) - S(tt)
                nc.vector.scalar_tensor_tensor(
                    out=trash[:], in0=tT[bii][:], scalar=1.0, in1=tTT[bii][:],
                    op0=Alu.mult, op1=Alu.max, accum_out=dcol(j, 4),
                ).then_inc(dve_sem, 1)
            # PSUM -> SBUF staging
            vector.wait_ge(pe_sem, 8 * T)
            nc.vector.tensor_copy(
                out=csum[0:1, 0:512], in_=psum_t[0:1, :]
            ).then_inc(dve_sem, 1)
            nc.vector.tensor_copy(
                out=csum[0:1, 512:1024], in_=psum_tt[0:1, :]
            ).then_inc(dve_sem, 1)

        @block.scalar
        def _(scalar):
            for jj in range(T):
                j = jj % NCHUNK
                bi = jj % NB
                bii = jj % NBI
                # |p| with free S(|p|), then exp(-|p|)
                scalar.wait_ge(dma_p, 16 * (jj + 1))
                nc.scalar.activation(
                    tAP[:], tP[bii][:], Act.Abs, accum_out=acol(j, 0)
                ).then_inc(act_sem, 1)
                nc.scalar.activation(
                    tE[:, 0:F], tAP[:], Act.Exp, scale=-1.0
                ).then_inc(act_sem, 1)
                # |a| with free S(|a|), then exp(-50|a|)
                scalar.wait_ge(dve_sem, 5 * jj + 1)  # a ready
                nc.scalar.activation(
                    tAA[:], tA[bi][:], Act.Abs, accum_out=acol(j, 1)
                ).then_inc(act_sem, 1)
                nc.scalar.activation(
                    tE[:, F : 2 * F], tAA[:], Act.Exp, scale=-R
                ).then_inc(act_sem, 1)
                nc.scalar.activation(
                    tF[:], tE[:], Act.Ln, bias=1.0, accum_out=acol(j, 2)
                ).then_inc(act_sem, 1)

        @block.tensor
        def _(tensor):
            for jj in range(T):
                bi = jj % NBI
                tensor.wait_ge(dma_t, 16 * (jj + 1))
                for k, (lo, hi) in enumerate(KSL):
                    w = hi - lo
                    nc.tensor.matmul(
                        psum_t[0:1, 0:w],
                        ones,
                        tT[bi][:, lo:hi],
                        start=(jj % NCHUNK == 0 and k == 0),
                        stop=(jj % NCHUNK == NCHUNK - 1 and k == 3),
                    ).then_inc(pe_sem, 1)
                tensor.wait_ge(dma_tt, 16 * (jj + 1))
                for k, (lo, hi) in enumerate(KSL):
                    w = hi - lo
                    nc.tensor.matmul(
                        psum_tt[0:1, 0:w],
                        ones,
                        tTT[bi][:, lo:hi],
                        start=(jj % NCHUNK == 0 and k == 0),
                        stop=(jj % NCHUNK == NCHUNK - 1 and k == 3),
                    ).then_inc(pe_sem, 1)

    _CACHE[nloop] = (nc, bass_utils)
    return _CACHE[nloop]


def _run_device(shards, **kwargs):
    """shards: dict name -> list of 8 [P, FDIM] f32 arrays."""
    nc, bass_utils = _build()
    in_maps = [
        {name: shards[name][c] for name in ("p", "t", "tp", "tt")}
        for c in range(N_CORES)
    ]
    return bass_utils.run_bass_kernel_spmd(
        nc, in_maps, core_ids=list(range(N_CORES)), **kwargs
    )


def _shard(arr):
    flat = np.ascontiguousarray(arr, dtype=np.float32).reshape(-1)
    return [
        flat[c * PER_CORE : (c + 1) * PER_CORE].reshape(P, FDIM)
        for c in range(N_CORES)
    ]


def _reduce_host(results):
    # acc_out: [0:20] DVE chunk-major (stt1=S(-p*tp), stt2a=S(-2500*a*tp),
    # stt2b=S(2500*a*tt), suma=S(a), smax=S(max(t,tt))), [20:32] ACT
    # chunk-major (absP=S(|p|), absA=S(|a|), lnC=S(ln1p_p)+S(ln1p_a)).
    # colsum_out: [0:512] S(t) cols, [512:1024] S(tt) cols.
    #   S(sp(p))   = 0.5 (S(a)+S(t)) + 0.5 S(|p|) + lnC_p
    #   S(sp(50a)) = 25 S(a) + 25 S(|a|) + lnC_a
    #   10 S(|c|)  = 20 S(max(t,tt)) - 10 S(tt) - 10 S(t)
    cd = np.array([1.0, 1.0, 1.0, 0.5 + R / 2.0, 2.0 * BETA])
    ca = np.array([0.5, R / 2.0, 1.0])  # absP, absA, lnC
    total = 0.0
    for c in range(N_CORES):
        out = results[c]["acc_out"].astype(np.float64)
        dve = out[:, : 5 * NCHUNK].reshape(P, NCHUNK, 5)
        act = out[:, 5 * NCHUNK :].reshape(P, NCHUNK, 3)
        total += float((dve.sum(axis=(0, 1)) * cd).sum())
        total += float((act.sum(axis=(0, 1)) * ca).sum())
        cs = results[c]["colsum_out"].astype(np.float64).reshape(1024)
        total += (0.5 - BETA) * cs[0:512].sum()  # S(t)
        total += -BETA * cs[512:1024].sum()  # S(tt)
    return np.float32(total / NTOT)


def _numpy_fallback(p, t, tp, tt):
    """Exact reference semantics in float32 numpy (only used if the top-k
    collapse precondition ever fails)."""

    def bce(x, tgt):
        return (
            np.maximum(x, 0.0) - x * tgt + np.log1p(np.exp(-np.abs(x)))
        ).astype(np.float32)

    def balanced(x, tgt):
        losses = bce(x, tgt).ravel()
        mask = tgt.ravel() > 0.5
        n_pos = int(mask.sum())
        n_neg_avail = mask.size - n_pos
        n_negative = min(n_neg_avail, K * n_pos)
        pos_sum = np.float32(losses[mask].sum())
        neg_sorted = np.sort(losses[~mask])[::-1]
        neg_sum = np.float32(neg_sorted[:n_negative].sum())
        return (pos_sum + neg_sum) / np.float32(n_pos + n_negative)

    bin_map = (R * (p - t)).astype(np.float32)
    target_bin = (R * (tp - tt)).astype(np.float32)
    ls = balanced(p, tp)
    lb = balanced(bin_map, target_bin)
    lt = np.abs(t - tt).mean(dtype=np.float32)
    return np.float32(ls + ALPHA * lb + BETA * lt)


def kernel(
    proba_map, thresh_map, target_proba_map, target_thresh_map
) -> np.ndarray:
    p = np.asarray(proba_map, dtype=np.float32)
    t = np.asarray(thresh_map, dtype=np.float32)
    tp = np.asarray(target_proba_map, dtype=np.float32)
    tt = np.asarray(target_thresh_map, dtype=np.float32)

    # The device kernel assumes the hard-negative top-k keeps every negative
    # (n_neg_avail <= K*n_pos for both BCE terms). Cheap host check; exact
    # fallback otherwise.
    npos1 = int(np.count_nonzero(tp > 0.5))
    d = (R * (tp - tt)).astype(np.float32)
    npos2 = int(np.count_nonzero(d > 0.5))
    if (tp.size - npos1) > K * npos1 or (d.size - npos2) > K * npos2:
        return _numpy_fallback(p, t, tp, tt)

    shards = {"p": _shard(p), "t": _shard(t), "tp": _shard(tp), "tt": _shard(tt)}
    res = _run_device(shards)
    return _reduce_host(res.results)


# revision 23
# speedup vs baseline: 1.5288x; 1.0176x over previous
"""Trainium2 Bass kernel for nn_DBLoss_11605001634022.

DBLoss = Ls + Lb + 10*Lt over four (16,640,640) f32 maps, where Ls/Lb are
"balanced" BCE-with-logits losses with hard-negative mining (keep the top
n_negative = min(n_neg_avail, 3*n_pos) negative losses) and
Lt = mean|thresh - target_thresh|.

For these inputs the targets are ~uniform, so n_neg_avail <= 3*n_pos by a
huge margin and the top-k keeps ALL negatives; each balanced BCE collapses
to a plain mean of the elementwise BCE losses. With
bce(x, t) = softplus(x) - x*t, the whole loss is one streaming reduction:

  loss = [ S(sp(p)) - S(p*tp) + S(sp(50*a)) - 2500*S(a*b) + 10*S(|c|) ] / N
  a = p - t,  b = tp - tt,  c = t - tt,   S = sum over all elements

The kernel verifies the collapse condition on the host (cheap) and falls
back to an exact numpy implementation if it ever fails.

The HW has no softplus ACT table, so softplus uses the relu identity
  S(sp(x)) = (S(x) + S(|x|))/2 + S(ln(1 + exp(-|x|)))
with exp/ln in the single `natural_log_exp_and_others` ACT table set (one
table load, no switches). Likewise
  S(|c|) = 2 S(relu(c)) - S(c) = 2 S(max(t,tt)) - S(tt) - S(t).

Sharded batch-parallel: 2 images/core across 8 cores; each core streams
its 13.1 MB once, in 4 pipelined [128,1600] chunks (4-deep buffered input
and intermediate tiles), raw Bass + manual semaphores (the
Tile layer's multi-wait sync is rejected by this walrus). GPSIMD is kept
idle on purpose: its elementwise ops measured ~5x slower in situ than DVE
under full SBUF port contention, and DVE ops measured ~2.2 us each in
situ, so work is spread DVE/ACT/PE evenly. Per-tensor DMA semaphores let
each consumer start as soon as the specific tensor it needs has landed.
Per chunk:
  DVE  (5 scalar_tensor_tensor ops, each with a free row-sum):
        a=(p*1)-t (+S(a)); (p*-1)*tp (+S); (a*-2500)*tp and (a*2500)*tt
        (+S, the expanded -2500*a*b term); (t*1) max tt (+S(max), for the
        relu identity S(relu(t-tt)) = S(max(t,tt)) - S(tt)).
  ACT  (4 passes): |p| and |50a| (each +row-sum, the latter via the Abs
        pre-scale) into one [128,3200] buffer; one merged exp(-x) pass;
        one merged ln(1+u) pass (+row-sum).
  PE   (8 fp32 ones-matmuls): column sums of t and tt accumulated in
        PSUM across chunks (exact, PE idle otherwise).
Row-sums land in per-engine stats tiles (no cross-engine SBUF write
granule sharing); PSUM column sums staged to SBUF at the end. Host applies
coefficients and the final division in float64.
"""

import numpy as np

N_CORES = 8
SHAPE = (16, 640, 640)
NTOT = SHAPE[0] * SHAPE[1] * SHAPE[2]
PER_CORE = NTOT // N_CORES  # 819200
P = 128
FDIM = PER_CORE // P  # 6400
NCHUNK = 4
F = FDIM // NCHUNK  # 1600
R = 50.0
ALPHA = 1.0
BETA = 10.0
K = 3

_CACHE = {}


def _get_concourse():
    try:
        import concourse.bass  # noqa: F401
    except ImportError:
        import sys

        sys.path.insert(0, "/opt/trn_rl_repo")
    import concourse.bass as bass
    import concourse.mybir as mybir
    from concourse import bass_utils

    return bass, mybir, bass_utils


def _build(nloop=1):
    """Build the bass program. nloop > 1 repeats the whole pipeline nloop
    times inside one NEFF (same result; used for dispatch-free timing)."""
    if nloop in _CACHE:
        return _CACHE[nloop]
    import contextlib

    bass, mybir, bass_utils = _get_concourse()
    f32 = mybir.dt.float32
    Alu = mybir.AluOpType
    Act = mybir.ActivationFunctionType

    nc = bass.Bass()
    dp = nc.dram_tensor("p", [P, FDIM], f32, kind="ExternalInput")
    dt_ = nc.dram_tensor("t", [P, FDIM], f32, kind="ExternalInput")
    dtp = nc.dram_tensor("tp", [P, FDIM], f32, kind="ExternalInput")
    dtt = nc.dram_tensor("tt", [P, FDIM], f32, kind="ExternalInput")
    dout = nc.dram_tensor("acc_out", [P, 8 * NCHUNK], f32, kind="ExternalOutput")
    dout2 = nc.dram_tensor("colsum_out", [1, 1024], f32, kind="ExternalOutput")

    NB = 4  # intermediate (tA) buffers
    NBI = 4  # input tile buffers
    KSL = [(lo, min(lo + 512, F)) for lo in range(0, F, 512)]
    NPE = 2 * len(KSL)  # matmuls per chunk
    T = nloop * NCHUNK

    ctx = contextlib.ExitStack()
    with ctx:
        sb = lambda name, shape: ctx.enter_context(
            nc.sbuf_tensor(name, shape, f32)
        )
        tP = [sb(f"tP{i}", [P, F]) for i in range(NBI)]
        tT = [sb(f"tT{i}", [P, F]) for i in range(NBI)]
        tTP = [sb(f"tTP{i}", [P, F]) for i in range(NBI)]
        tTT = [sb(f"tTT{i}", [P, F]) for i in range(NBI)]
        tA = [sb(f"tA{i}", [P, F]) for i in range(NB)]
        tG = sb("tG", [P, 2 * F])  # [ |p| | |50a| ]
        tE = sb("tE", [P, 2 * F])  # exp outputs (p-half | a-half)
        tF = sb("tF", [P, 2 * F])  # ln dump
        trash = sb("trash", [P, 1])
        acc_d = sb("acc_d", [P, 5 * NCHUNK])
        acc_a = sb("acc_a", [P, 3 * NCHUNK])  # absP, absA, lnC
        csum = sb("csum", [1, 1024])
        psum_t = ctx.enter_context(nc.psum_tensor("psum_t", [1, 512], f32))
        psum_tt = ctx.enter_context(nc.psum_tensor("psum_tt", [1, 512], f32))

        ones = nc.const_aps.tensor(1.0, (P, 1), f32)

        dma_p = ctx.enter_context(nc.semaphore())
        dma_t = ctx.enter_context(nc.semaphore())
        dma_tp = ctx.enter_context(nc.semaphore())
        dma_tt = ctx.enter_context(nc.semaphore())
        dve_sem = ctx.enter_context(nc.semaphore())
        act_sem = ctx.enter_context(nc.semaphore())
        pe_sem = ctx.enter_context(nc.semaphore())
        block = ctx.enter_context(nc.Block())

        def dcol(j, k):
            return acc_d[:, 5 * j + k : 5 * j + k + 1]

        def acol(j, k):
            return acc_a[:, 3 * j + k : 3 * j + k + 1]

        @block.sync
        def _(sync):
            for jj in range(T):
                j = jj % NCHUNK
                bi = jj % NBI
                sl = slice(j * F, (j + 1) * F)
                if jj >= NBI:
                    # input buffers of chunk jj-3 must be fully consumed
                    sync.wait_ge(dve_sem, 5 * (jj - 2))
                    sync.wait_ge(act_sem, 4 * (jj - 3) + 1)  # absP read tP
                    sync.wait_ge(pe_sem, NPE * (jj - 2))
                sync.dma_start(out=tP[bi][:], in_=dp[:, sl]).then_inc(dma_p, 16)
                sync.dma_start(out=tT[bi][:], in_=dt_[:, sl]).then_inc(dma_t, 16)
                sync.dma_start(out=tTP[bi][:], in_=dtp[:, sl]).then_inc(dma_tp, 16)
                sync.dma_start(out=tTT[bi][:], in_=dtt[:, sl]).then_inc(dma_tt, 16)
            sync.wait_ge(dve_sem, 5 * T + 2)  # incl. PSUM->SBUF copies
            sync.wait_ge(act_sem, 4 * T)
            sync.dma_start(
                out=dout[:, : 5 * NCHUNK], in_=acc_d[:]
            ).then_inc(dma_p, 16)
            sync.dma_start(
                out=dout[:, 5 * NCHUNK :], in_=acc_a[:]
            ).then_inc(dma_p, 16)
            sync.dma_start(out=dout2[:], in_=csum[:]).then_inc(dma_p, 16)
            sync.wait_ge(dma_p, 16 * T + 48)
            sync.wait_ge(dma_t, 16 * T)
            sync.wait_ge(dma_tp, 16 * T)
            sync.wait_ge(dma_tt, 16 * T)

        @block.vector
        def _(vector):
            for jj in range(T):
                j = jj % NCHUNK
                bi = jj % NB
                bii = jj % NBI
                vector.wait_ge(dma_p, 16 * (jj + 1))
                if jj >= NB:
                    # absA of chunk jj-3 must have read tA[bi]
                    vector.wait_ge(act_sem, 4 * (jj - NB) + 2)
                # a = p - t, with free S(a)
                vector.wait_ge(dma_t, 16 * (jj + 1))
                nc.vector.scalar_tensor_tensor(
                    out=tA[bi][:], in0=tP[bii][:], scalar=1.0, in1=tT[bii][:],
                    op0=Alu.mult, op1=Alu.subtract, accum_out=dcol(j, 3),
                ).then_inc(dve_sem, 1)
                # S(-p*tp)
                vector.wait_ge(dma_tp, 16 * (jj + 1))
                nc.vector.scalar_tensor_tensor(
                    out=trash.broadcast_to((P, F)), in0=tP[bii][:], scalar=-1.0, in1=tTP[bii][:],
                    op0=Alu.mult, op1=Alu.mult, accum_out=dcol(j, 0),
                ).then_inc(dve_sem, 1)
                # -2500*S(a*b) expanded: S(-2500*a*tp) + S(2500*a*tt)
                nc.vector.scalar_tensor_tensor(
                    out=trash.broadcast_to((P, F)), in0=tA[bi][:], scalar=-2500.0, in1=tTP[bii][:],
                    op0=Alu.mult, op1=Alu.mult, accum_out=dcol(j, 1),
                ).then_inc(dve_sem, 1)
                vector.wait_ge(dma_tt, 16 * (jj + 1))
                nc.vector.scalar_tensor_tensor(
                    out=trash.broadcast_to((P, F)), in0=tA[bi][:], scalar=2500.0, in1=tTT[bii][:],
                    op0=Alu.mult, op1=Alu.mult, accum_out=dcol(j, 2),
                ).then_inc(dve_sem, 1)
                # S(max(t,tt)): S(relu(t-tt)) = S(max) - S(tt)
                nc.vector.scalar_tensor_tensor(
                    out=trash.broadcast_to((P, F)), in0=tT[bii][:], scalar=1.0, in1=tTT[bii][:],
                    op0=Alu.mult, op1=Alu.max, accum_out=dcol(j, 4),
                ).then_inc(dve_sem, 1)
            # PSUM -> SBUF staging
            vector.wait_ge(pe_sem, NPE * T)
            nc.vector.tensor_copy(
                out=csum[0:1, 0:512], in_=psum_t[0:1, :]
            ).then_inc(dve_sem, 1)
            nc.vector.tensor_copy(
                out=csum[0:1, 512:1024], in_=psum_tt[0:1, :]
            ).then_inc(dve_sem, 1)

        @block.scalar
        def _(scalar):
            for jj in range(T):
                j = jj % NCHUNK
                bi = jj % NB
                bii = jj % NBI
                # |p| with free S(|p|)
                scalar.wait_ge(dma_p, 16 * (jj + 1))
                nc.scalar.activation(
                    tG[:, 0:F], tP[bii][:], Act.Abs, accum_out=acol(j, 0)
                ).then_inc(act_sem, 1)
                # |50a| with free S(|50a|)
                scalar.wait_ge(dve_sem, 5 * jj + 1)  # a ready
                nc.scalar.activation(
                    tG[:, F : 2 * F], tA[bi][:], Act.Abs, scale=R,
                    accum_out=acol(j, 1),
                ).then_inc(act_sem, 1)
                # exp(-|p|) | exp(-|50a|) in one pass
                nc.scalar.activation(
                    tE[:], tG[:], Act.Exp, scale=-1.0
                ).then_inc(act_sem, 1)
                nc.scalar.activation(
                    tF[:], tE[:], Act.Ln, bias=1.0, accum_out=acol(j, 2)
                ).then_inc(act_sem, 1)

        @block.tensor
        def _(tensor):
            for jj in range(T):
                bi = jj % NBI
                tensor.wait_ge(dma_t, 16 * (jj + 1))
                for k, (lo, hi) in enumerate(KSL):
                    w = hi - lo
                    nc.tensor.matmul(
                        psum_t[0:1, 0:w],
                        ones,
                        tT[bi][:, lo:hi],
                        start=(jj % NCHUNK == 0 and k == 0),
                        stop=(jj % NCHUNK == NCHUNK - 1 and k == len(KSL) - 1),
                    ).then_inc(pe_sem, 1)
                tensor.wait_ge(dma_tt, 16 * (jj + 1))
                for k, (lo, hi) in enumerate(KSL):
                    w = hi - lo
                    nc.tensor.matmul(
                        psum_tt[0:1, 0:w],
                        ones,
                        tTT[bi][:, lo:hi],
                        start=(jj % NCHUNK == 0 and k == 0),
                        stop=(jj % NCHUNK == NCHUNK - 1 and k == len(KSL) - 1),
                    ).then_inc(pe_sem, 1)

    _CACHE[nloop] = (nc, bass_utils)
    return _CACHE[nloop]


def _run_device(shards, **kwargs):
    """shards: dict name -> list of 8 [P, FDIM] f32 arrays."""
    nc, bass_utils = _build()
    in_maps = [
        {name: shards[name][c] for name in ("p", "t", "tp", "tt")}
        for c in range(N_CORES)
    ]
    return bass_utils.run_bass_kernel_spmd(
        nc, in_maps, core_ids=list(range(N_CORES)), **kwargs
    )


def _shard(arr):
    flat = np.ascontiguousarray(arr, dtype=np.float32).reshape(-1)
    return [
        flat[c * PER_CORE : (c + 1) * PER_CORE].reshape(P, FDIM)
        for c in range(N_CORES)
    ]


def _reduce_host(results):
    # acc_out: [0:20] DVE chunk-major (stt1=S(-p*tp), stt2a=S(-2500*a*tp),
    # stt2b=S(2500*a*tt), suma=S(a), smax=S(max(t,tt))), [20:32] ACT
    # chunk-major (absP=S(|p|), absA=S(|a|), lnC=S(ln1p_p)+S(ln1p_a)).
    # colsum_out: [0:512] S(t) cols, [512:1024] S(tt) cols.
    #   S(sp(p))   = 0.5 (S(a)+S(t)) + 0.5 S(|p|) + lnC_p
    #   S(sp(50a)) = 25 S(a) + 25 S(|a|) + lnC_a
    #   10 S(|c|)  = 20 S(max(t,tt)) - 10 S(tt) - 10 S(t)
    cd = np.array([1.0, 1.0, 1.0, 0.5 + R / 2.0, 2.0 * BETA])
    ca = np.array([0.5, 0.5, 1.0])  # S(|p|), S(|50a|), lnC
    total = 0.0
    for c in range(N_CORES):
        out = results[c]["acc_out"].astype(np.float64)
        dve = out[:, : 5 * NCHUNK].reshape(P, NCHUNK, 5)
        act = out[:, 5 * NCHUNK :].reshape(P, NCHUNK, 3)
        total += float((dve.sum(axis=(0, 1)) * cd).sum())
        total += float((act.sum(axis=(0, 1)) * ca).sum())
        cs = results[c]["colsum_out"].astype(np.float64).reshape(1024)
        total += (0.5 - BETA) * cs[0:512].sum()  # S(t)
        total += -BETA * cs[512:1024].sum()  # S(tt)
    return np.float32(total / NTOT)


def _numpy_fallback(p, t, tp, tt):
    """Exact reference semantics in float32 numpy (only used if the top-k
    collapse precondition ever fails)."""

    def bce(x, tgt):
        return (
            np.maximum(x, 0.0) - x * tgt + np.log1p(np.exp(-np.abs(x)))
        ).astype(np.float32)

    def balanced(x, tgt):
        losses = bce(x, tgt).ravel()
        mask = tgt.ravel() > 0.5
        n_pos = int(mask.sum())
        n_neg_avail = mask.size - n_pos
        n_negative = min(n_neg_avail, K * n_pos)
        pos_sum = np.float32(losses[mask].sum())
        neg_sorted = np.sort(losses[~mask])[::-1]
        neg_sum = np.float32(neg_sorted[:n_negative].sum())
        return (pos_sum + neg_sum) / np.float32(n_pos + n_negative)

    bin_map = (R * (p - t)).astype(np.float32)
    target_bin = (R * (tp - tt)).astype(np.float32)
    ls = balanced(p, tp)
    lb = balanced(bin_map, target_bin)
    lt = np.abs(t - tt).mean(dtype=np.float32)
    return np.float32(ls + ALPHA * lb + BETA * lt)


def kernel(
    proba_map, thresh_map, target_proba_map, target_thresh_map
) -> np.ndarray:
    p = np.asarray(proba_map, dtype=np.float32)
    t = np.asarray(thresh_map, dtype=np.float32)
    tp = np.asarray(target_proba_map, dtype=np.float32)
    tt = np.asarray(target_thresh_map, dtype=np.float32)

    # The device kernel assumes the hard-negative top-k keeps every negative
    # (n_neg_avail <= K*n_pos for both BCE terms). Cheap host check; exact
    # fallback otherwise.
    npos1 = int(np.count_nonzero(tp > 0.5))
    d = (R * (tp - tt)).astype(np.float32)
    npos2 = int(np.count_nonzero(d > 0.5))
    if (tp.size - npos1) > K * npos1 or (d.size - npos2) > K * npos2:
        return _numpy_fallback(p, t, tp, tt)

    shards = {"p": _shard(p), "t": _shard(t), "tp": _shard(tp), "tt": _shard(tt)}
    res = _run_device(shards)
    return _reduce_host(res.results)
